# revision 1
# baseline (speedup 1.0000x reference)
"""Bass/Trainium2 kernel for nn_BivariateSpectral: batched smallest-eigenvalue of
S_b = sym(A + B*diag(x_b) + C*diag(y_b)), b = 0..32767, each 64x64, 8 NeuronCores.

Algorithm (per core, data-parallel over batch):
  Phase 1 - batched Lanczos (K steps) on D_b = (M_b + M_b^T)/64 = S_b/32.
    Key identity: the batched matvec over all b is shared 64x64 matmuls:
      D v = Ah v + Bh (x*v) + Ch (y*v) + Ah^T v + x*(Bh^T v) + y*(Ch^T v)
    Layout: dim on partitions (two batch-halves packed as partitions 0-63 /
    64-127 with block-diagonal stationaries), batch on the free dim.
    Per-batch-column alpha_j / beta_j^2 extracted via ones-block-diag matmuls
    (partition reduction + full-partition broadcast on the PE).
  Phase 2 - Sturm bisection on the K x K tridiagonals, batch on partitions,
    division-free char-poly recurrence, 4 shifts/pass; eigenvalue eigval_idx
    via count<=idx targeting.  Output scaled back by 32.
"""

import functools
import numpy as np

BATCH, DIM = 32768, 64
NCORES = 8
SHARD = BATCH // NCORES      # 4096 batch elems per core
NFREE = SHARD // 2           # 2048 free columns (two partition-halves)
CHUNK = 1024                 # psum chunk (2 banks)
NCH = NFREE // CHUNK         # 2
K = 34                       # Lanczos steps
NB = K - 1                   # number of betas
ROWS_A = 2 * K               # 72 rows in TA staging (2j+h)
ROWS_B = 2 * NB              # 70 rows in TB staging
TG = NFREE // 128            # 16 transpose column-groups
NS = 4                       # bisection shifts per pass
PASSES = 6
C_OP = np.float32(1.0 / 64.0)   # A,B,C host prescale: D = (M+M^T)/64 = S/32
OUT_SCALE = 16.0                # lam_S = 32 * 0.5 * (lo+hi)


def _v0_vec():
    rng = np.random.default_rng(1234)
    v = rng.standard_normal(DIM).astype(np.float64)
    v /= np.sqrt((v * v).sum())
    return v.astype(np.float32)


def _bd(m):
    """128x128 block-diagonal duplication of a 64x64 matrix."""
    out = np.zeros((128, 128), np.float32)
    out[:64, :64] = m
    out[64:, 64:] = m
    return out


def _bcast_s(ap, extra_off=0, count=2, ns=NS):
    """Insert a 0-step 'shift' dim after the partition dim of a [128, T, R] AP,
    slicing 'count' elems at free offset extra_off: -> [128, ns, T, count]."""
    import concourse.bass as bass
    dims = list(ap.ap)
    part = dims[0]
    tdim = dims[1]
    return bass.AP(
        tensor=ap.tensor,
        offset=ap.offset + extra_off,
        ap=[part, [0, ns], tdim, [1, count]],
    )


def _bcast_flat(ap, ns=NS):
    """[128, T, 2] AP -> [128, ns, T, 2] via 0-step shift dim."""
    import concourse.bass as bass
    dims = list(ap.ap)
    return bass.AP(tensor=ap.tensor, offset=ap.offset, ap=[dims[0], [0, ns]] + dims[1:])


@functools.lru_cache(maxsize=4)
def _program(idx: int):
    import concourse.bacc as bacc
    import concourse.bass as bass
    import concourse.mybir as mybir
    import concourse.tile as tile
    from concourse.masks import make_identity

    F32 = mybir.dt.float32
    F32R = mybir.dt.float32r
    I32 = mybir.dt.int32
    OP = mybir.AluOpType
    ACTF = mybir.ActivationFunctionType

    nc = bacc.Bacc("TRN2", target_bir_lowering=False, debug=False)

    x_in = nc.dram_tensor("x", [128, NFREE], F32, kind="ExternalInput").ap()
    y_in = nc.dram_tensor("y", [128, NFREE], F32, kind="ExternalInput").ap()
    lms_in = nc.dram_tensor("lms", [128, 128], F32, kind="ExternalInput").ap()
    lbf_in = nc.dram_tensor("lbf", [128, 128], F32, kind="ExternalInput").ap()
    lcf_in = nc.dram_tensor("lcf", [128, 128], F32, kind="ExternalInput").ap()
    lbt_in = nc.dram_tensor("lbt", [128, 128], F32, kind="ExternalInput").ap()
    lct_in = nc.dram_tensor("lct", [128, 128], F32, kind="ExternalInput").ap()
    obd_in = nc.dram_tensor("obd", [128, 128], F32, kind="ExternalInput").ap()
    v0_in = nc.dram_tensor("v0", [128, 1], F32, kind="ExternalInput").ap()
    lam_out = nc.dram_tensor("lam", [SHARD], F32, kind="ExternalOutput").ap()

    ta_dram = nc.dram_tensor("ta_stage", [ROWS_A, NFREE], F32).ap()
    tb_dram = nc.dram_tensor("tb_stage", [ROWS_B, NFREE], F32).ap()

    with tile.TileContext(nc) as tc:
        # ---------------- Phase 1: Lanczos ----------------
        with (
            tc.tile_pool(name="singles", bufs=1) as singles,
            tc.tile_pool(name="vpool", bufs=3) as vpool,
            tc.tile_pool(name="work", bufs=1) as work,
            tc.tile_pool(name="bbp", bufs=2) as bbp,
            tc.tile_pool(name="rows", bufs=1) as rowsp,
            tc.tile_pool(name="pw", bufs=2, space="PSUM") as pwp,
            tc.tile_pool(name="p3", bufs=2, space="PSUM") as p3p,
            tc.tile_pool(name="p4", bufs=2, space="PSUM") as p4p,
            tc.tile_pool(name="pbc", bufs=2, space="PSUM") as pbcp,
        ):
            xt = singles.tile([128, NFREE], F32)
            yt = singles.tile([128, NFREE], F32)
            nc.sync.dma_start(out=xt[:], in_=x_in)
            nc.sync.dma_start(out=yt[:], in_=y_in)
            lms = singles.tile([128, 128], F32)
            lbf = singles.tile([128, 128], F32)
            lcf = singles.tile([128, 128], F32)
            lbt = singles.tile([128, 128], F32)
            lct = singles.tile([128, 128], F32)
            obd = singles.tile([128, 128], F32)
            nc.sync.dma_start(out=lms[:], in_=lms_in)
            nc.sync.dma_start(out=lbf[:], in_=lbf_in)
            nc.sync.dma_start(out=lcf[:], in_=lcf_in)
            nc.sync.dma_start(out=lbt[:], in_=lbt_in)
            nc.sync.dma_start(out=lct[:], in_=lct_in)
            nc.sync.dma_start(out=obd[:], in_=obd_in)
            lms_r = singles.tile([128, 128], F32R)
            lbf_r = singles.tile([128, 128], F32R)
            lcf_r = singles.tile([128, 128], F32R)
            lbt_r = singles.tile([128, 128], F32R)
            lct_r = singles.tile([128, 128], F32R)
            obd_r = singles.tile([128, 128], F32R)
            nc.vector.tensor_copy(lms_r[:], lms[:])
            nc.vector.tensor_copy(lbf_r[:], lbf[:])
            nc.vector.tensor_copy(lcf_r[:], lcf[:])
            nc.vector.tensor_copy(lbt_r[:], lbt[:])
            nc.vector.tensor_copy(lct_r[:], lct[:])
            nc.vector.tensor_copy(obd_r[:], obd[:])
            v0t = singles.tile([128, 1], F32)
            nc.sync.dma_start(out=v0t[:], in_=v0_in)
            epst = singles.tile([128, 1], F32)
            nc.vector.memset(epst[:], 1e-12)

            # Two independent batch groups (columns [0,1024) and [1024,2048))
            # run interleaved Lanczos recursions so one group's dependency
            # chain hides behind the other's work.
            st = []
            for g in range(NCH):
                v_cur = vpool.tile([128, CHUNK], F32R, tag=f"v{g}")
                nc.vector.tensor_copy(v_cur[:],
                                      v0t[:, 0:1].to_broadcast((128, CHUNK)))
                st.append({"v": v_cur, "vp": None, "bb": None})

            for j in range(K):
                last = j == K - 1
                T = [{} for _ in range(NCH)]
                # ---- phase A: front muls (gpsimd) + matvec matmuls (PE) ----
                for g in range(NCH):
                    S, D = st[g], T[g]
                    v_cur, v_prev, bb_prev = S["v"], S["vp"], S["bb"]
                    D["w"] = work.tile([128, CHUNK], F32, tag=f"w{g}", name=f"w{g}")
                    D["t1"] = work.tile([128, CHUNK], F32R, tag=f"t1{g}", name=f"t1{g}")
                    D["t2"] = work.tile([128, CHUNK], F32R, tag=f"t2{g}", name=f"t2{g}")
                    if j > 0 and not last:
                        D["t4"] = work.tile([128, CHUNK], F32, tag=f"t4{g}", name=f"t4{g}")
                        nc.gpsimd.tensor_mul(D["t4"][:], bb_prev[:], v_prev[:])
                    nc.gpsimd.tensor_mul(D["t1"][:], xt[:, g * CHUNK:(g + 1) * CHUNK],
                                         v_cur[:])
                    nc.gpsimd.tensor_mul(D["t2"][:], yt[:, g * CHUNK:(g + 1) * CHUNK],
                                         v_cur[:])
                for g in range(NCH):
                    S, D = st[g], T[g]
                    v_cur = S["v"]
                    D["pw"], D["p3"], D["p4"] = [], [], []
                    for n0 in range(0, CHUNK, 512):
                        ns = slice(n0, n0 + 512)
                        pw = pwp.tile([128, 512], F32, tag="pw")
                        p3 = p3p.tile([128, 512], F32, tag="p3")
                        p4 = p4p.tile([128, 512], F32, tag="p4")
                        D["pw"].append(pw), D["p3"].append(p3), D["p4"].append(p4)
                        nc.tensor.matmul(pw[:], lms_r[:], v_cur[:, ns],
                                         start=True, stop=False)
                        nc.tensor.matmul(pw[:], lbf_r[:], D["t1"][:, ns],
                                         start=False, stop=False)
                        nc.tensor.matmul(pw[:], lcf_r[:], D["t2"][:, ns],
                                         start=False, stop=True)
                        nc.tensor.matmul(p3[:], lbt_r[:], v_cur[:, ns],
                                         start=True, stop=True)
                        nc.tensor.matmul(p4[:], lct_r[:], v_cur[:, ns],
                                         start=True, stop=True)
                # ---- phase B: combine (DVE), p (gps), alpha bcast (PE), rows ----
                for g in range(NCH):
                    S, D = st[g], T[g]
                    w = D["w"]
                    for i, n0 in enumerate(range(0, CHUNK, 512)):
                        ns = slice(n0, n0 + 512)
                        xs = slice(g * CHUNK + n0, g * CHUNK + n0 + 512)
                        m1 = work.tile([128, 512], F32, tag=f"m1{g}")
                        m2 = work.tile([128, 512], F32, tag=f"m2{g}")
                        nc.vector.tensor_mul(m1[:], xt[:, xs], D["p3"][i][:])
                        nc.vector.tensor_add(w[:, ns], D["pw"][i][:], m1[:])
                        nc.vector.tensor_mul(m2[:], yt[:, xs], D["p4"][i][:])
                        nc.vector.tensor_add(w[:, ns], w[:, ns], m2[:])
                for g in range(NCH):
                    S, D = st[g], T[g]
                    p_t = work.tile([128, CHUNK], F32R, tag=f"pq{g}")
                    nc.gpsimd.tensor_mul(p_t[:], S["v"][:], D["w"][:])
                    D["p_t"] = p_t
                for g in range(NCH):
                    D = T[g]
                    D["ab"] = []
                    for n0 in range(0, CHUNK, 512):
                        ab = pbcp.tile([128, 512], F32, tag="pbc")
                        D["ab"].append(ab)
                        nc.tensor.matmul(ab[:], obd_r[:], D["p_t"][:, n0 : n0 + 512],
                                         start=True, stop=True)
                # ---- phase C: orthogonalize + beta + normalize + stage rows ----
                for g in range(NCH):
                    S, D = st[g], T[g]
                    w, v_cur = D["w"], S["v"]
                    ra0 = rowsp.tile([1, CHUNK], F32, tag=f"ra0{g}")
                    ra1 = rowsp.tile([1, CHUNK], F32, tag=f"ra1{g}")
                    D["ra0"], D["ra1"] = ra0, ra1
                    for i, n0 in enumerate(range(0, CHUNK, 512)):
                        ns = slice(n0, n0 + 512)
                        nc.scalar.activation(ra0[0:1, ns], D["ab"][i][0:1, :],
                                             ACTF.Copy)
                        nc.scalar.activation(ra1[0:1, ns], D["ab"][i][64:65, :],
                                             ACTF.Copy)
                    if not last:
                        for i, n0 in enumerate(range(0, CHUNK, 512)):
                            ns = slice(n0, n0 + 512)
                            t3 = work.tile([128, 512], F32, tag=f"m1{g}")
                            nc.vector.tensor_mul(t3[:], D["ab"][i][:], v_cur[:, ns])
                            nc.vector.tensor_sub(w[:, ns], w[:, ns], t3[:])
                        if j > 0:
                            nc.vector.tensor_sub(w[:], w[:], D["t4"][:])
                for g in range(NCH):
                    S, D = st[g], T[g]
                    if not last:
                        q_t = work.tile([128, CHUNK], F32R, tag=f"pq{g}")
                        nc.gpsimd.tensor_mul(q_t[:], D["w"][:], D["w"][:])
                        D["q_t"] = q_t
                for g in range(NCH):
                    S, D = st[g], T[g]
                    w = D["w"]
                    csl = slice(g * CHUNK, (g + 1) * CHUNK)
                    if not last:
                        rb0 = rowsp.tile([1, CHUNK], F32, tag=f"rb0{g}")
                        rb1 = rowsp.tile([1, CHUNK], F32, tag=f"rb1{g}")
                        bb = bbp.tile([128, CHUNK], F32, tag=f"bb{g}")
                        rb = work.tile([128, CHUNK], F32, tag=f"rb{g}")
                        v_nxt = vpool.tile([128, CHUNK], F32R, tag=f"v{g}")
                        b2l = []
                        for n0 in range(0, CHUNK, 512):
                            b2 = pbcp.tile([128, 512], F32, tag="pbc")
                            b2l.append(b2)
                            nc.tensor.matmul(b2[:], obd_r[:],
                                             D["q_t"][:, n0 : n0 + 512],
                                             start=True, stop=True)
                        for i, n0 in enumerate(range(0, CHUNK, 512)):
                            ns = slice(n0, n0 + 512)
                            nc.scalar.activation(bb[:, ns], b2l[i][:], ACTF.Sqrt,
                                                 bias=epst[:], scale=1.0)
                        nc.vector.reciprocal_approx_fast(out=rb[:], in_=bb[:])
                        nc.vector.tensor_mul(v_nxt[:], w[:], rb[:])
                        for i, n0 in enumerate(range(0, CHUNK, 512)):
                            ns = slice(n0, n0 + 512)
                            nc.scalar.activation(rb0[0:1, ns], b2l[i][0:1, :],
                                                 ACTF.Copy)
                            nc.scalar.activation(rb1[0:1, ns], b2l[i][64:65, :],
                                                 ACTF.Copy)
                        nc.sync.dma_start(out=tb_dram[2 * j : 2 * j + 1, csl],
                                          in_=rb0[:])
                        nc.sync.dma_start(out=tb_dram[2 * j + 1 : 2 * j + 2, csl],
                                          in_=rb1[:])
                        S["vp"] = S["v"]
                        S["v"] = v_nxt
                        S["bb"] = bb
                    nc.sync.dma_start(out=ta_dram[2 * j : 2 * j + 1, csl],
                                      in_=D["ra0"][:])
                    nc.sync.dma_start(out=ta_dram[2 * j + 1 : 2 * j + 2, csl],
                                      in_=D["ra1"][:])

        # ---------------- Phase 2: transpose + Sturm bisection ----------------
        with (
            tc.tile_pool(name="bis", bufs=1) as bis,
            tc.tile_pool(name="chk", bufs=2) as chk,
            tc.tile_pool(name="st3", bufs=1) as st3,
            tc.tile_pool(name="pt", bufs=2, space="PSUM") as ptp,
        ):
            ident = bis.tile([128, 128], F32)
            make_identity(nc, ident[:])

            td_a = bis.tile([128, TG, ROWS_A], F32)
            td_b = bis.tile([128, TG, ROWS_B], F32)
            for t in range(TG):
                csl = slice(t * 128, (t + 1) * 128)
                ca = chk.tile([ROWS_A, 128], F32, tag="chka")
                nc.sync.dma_start(out=ca[:], in_=ta_dram[:, csl])
                pa = ptp.tile([128, ROWS_A], F32, tag="pt")
                nc.tensor.transpose(pa[:], ca[:], ident[0:ROWS_A, 0:ROWS_A])
                nc.vector.tensor_copy(td_a[:, t, :], pa[:])
                cb = chk.tile([ROWS_B, 128], F32, tag="chkb")
                nc.sync.dma_start(out=cb[:], in_=tb_dram[:, csl])
                pb = ptp.tile([128, ROWS_B], F32, tag="pt")
                nc.tensor.transpose(pb[:], cb[:], ident[0:ROWS_B, 0:ROWS_B])
                nc.vector.tensor_copy(td_b[:, t, :], pb[:])

            import concourse.bass as bass_mod

            def jdims_ap(tile_ap, nj, step0=2):
                """[128, TG, R] AP viewed as [128, TG, 2, nj] with j innermost."""
                d = list(tile_ap.ap)
                return bass_mod.AP(
                    tensor=tile_ap.tensor, offset=tile_ap.offset,
                    ap=[d[0], d[1], [1, 2], [step0, nj]],
                )

            # |beta_j| for Gershgorin
            absb = bis.tile([128, TG, ROWS_B], F32)
            nc.scalar.activation(absb[:], td_b[:], ACTF.Sqrt)
            g = bis.tile([128, TG, ROWS_A], F32)
            nc.vector.tensor_copy(g[:], td_a[:])
            nc.vector.tensor_sub(g[:, :, 2:ROWS_A], g[:, :, 2:ROWS_A], absb[:])
            nc.vector.tensor_sub(g[:, :, 0:ROWS_B], g[:, :, 0:ROWS_B], absb[:])

            lo = bis.tile([128, TG, 2], F32)
            hi = bis.tile([128, TG, 2], F32)
            nc.vector.tensor_reduce(lo[:], jdims_ap(g[:], K), mybir.AxisListType.X,
                                    OP.min)
            if idx == 0:
                nc.vector.tensor_reduce(hi[:], jdims_ap(td_a[:], K),
                                        mybir.AxisListType.X, OP.min)
            else:
                g2 = g
                nc.vector.tensor_copy(g2[:], td_a[:])
                nc.vector.tensor_add(g2[:, :, 2:ROWS_A], g2[:, :, 2:ROWS_A], absb[:])
                nc.vector.tensor_add(g2[:, :, 0:ROWS_B], g2[:, :, 0:ROWS_B], absb[:])
                nc.vector.tensor_reduce(hi[:], jdims_ap(g2[:], K),
                                        mybir.AxisListType.X, OP.max)

            cs = bis.tile([128, NS, TG, 2], F32)
            for s in range(NS):
                nc.vector.memset(cs[:, s, :, :], float(s + 1) / float(NS + 1))

            sig = bis.tile([128, NS, TG, 2], F32)
            d_t = bis.tile([128, TG, 2], F32)
            pA = st3.tile([128, NS, TG, 2], F32, tag="pA")
            pB = st3.tile([128, NS, TG, 2], F32, tag="pB")
            pC = st3.tile([128, NS, TG, 2], F32, tag="pC")
            cA = st3.tile([128, NS, TG, 2], F32, tag="cA")
            cB = st3.tile([128, NS, TG, 2], F32, tag="cB")
            ca_t = st3.tile([128, NS, TG, 2], F32, tag="ca")
            u_t = st3.tile([128, NS, TG, 2], F32, tag="u")
            tb_t = st3.tile([128, NS, TG, 2], F32, tag="tb")
            sg_t = st3.tile([128, NS, TG, 2], F32, tag="sg")
            mle = bis.tile([128, TG, 2], I32)
            mgt = bis.tile([128, TG, 2], I32)

            thr = float(idx) + 0.5
            for ip in range(PASSES):
                nc.vector.tensor_sub(d_t[:], hi[:], lo[:])
                nc.vector.tensor_mul(sig[:], cs[:], _bcast_flat(d_t[:]))
                nc.vector.tensor_add(sig[:], sig[:], _bcast_flat(lo[:]))
                po, pp, pn = pA, pB, pC
                nc.vector.memset(po[:], 1.0)
                nc.vector.tensor_sub(pp[:], _bcast_s(td_a[:], 0), sig[:])
                cnt, cnt_nxt = cA, cB
                nc.vector.tensor_scalar(out=cnt[:], in0=pp[:], scalar1=0.0,
                                        scalar2=None, op0=OP.is_lt)
                for j in range(1, K):
                    nc.vector.tensor_sub(ca_t[:], _bcast_s(td_a[:], 2 * j), sig[:])
                    nc.vector.tensor_mul(u_t[:], ca_t[:], pp[:])
                    nc.vector.tensor_mul(tb_t[:], _bcast_s(td_b[:], 2 * (j - 1)),
                                         po[:])
                    nc.vector.tensor_sub(pn[:], u_t[:], tb_t[:])
                    nc.vector.tensor_mul(sg_t[:], pn[:], pp[:])
                    nc.vector.scalar_tensor_tensor(
                        out=cnt_nxt[:], in0=sg_t[:], scalar=0.0, in1=cnt[:],
                        op0=OP.is_lt, op1=OP.add)
                    po, pp, pn = pp, pn, po
                    cnt, cnt_nxt = cnt_nxt, cnt
                for s in range(NS):
                    nc.vector.tensor_scalar(out=mle[:], in0=cnt[:, s, :, :],
                                            scalar1=thr, scalar2=None, op0=OP.is_le)
                    nc.vector.copy_predicated(out=lo[:], mask=mle[:],
                                              data=sig[:, s, :, :])
                for s in range(NS - 1, -1, -1):
                    nc.vector.tensor_scalar(out=mgt[:], in0=cnt[:, s, :, :],
                                            scalar1=thr, scalar2=None, op0=OP.is_gt)
                    nc.vector.copy_predicated(out=hi[:], mask=mgt[:],
                                              data=sig[:, s, :, :])

            lam_t = bis.tile([128, TG, 2], F32)
            nc.vector.tensor_add(lam_t[:], lo[:], hi[:])
            nc.vector.tensor_scalar(out=lam_t[:], in0=lam_t[:], scalar1=OUT_SCALE,
                                    scalar2=None, op0=OP.mult)
            lam_ap = lam_out.rearrange("(h t p) -> h p t", h=2, t=TG, p=128)
            for h in range(2):
                nc.sync.dma_start(out=lam_ap[h], in_=lam_t[:, :, h])

    nc.compile()
    return nc


def kernel(x, y, A, B, C, eigval_idx):
    from concourse.bass_utils import run_bass_kernel_spmd

    idx = int(np.asarray(eigval_idx))
    nc = _program(idx)

    A32 = np.asarray(A, np.float32) * C_OP
    B32 = np.asarray(B, np.float32) * C_OP
    C32 = np.asarray(C, np.float32) * C_OP
    lms = _bd(A32 + A32.T)
    lbf = _bd(B32.T)
    lcf = _bd(C32.T)
    lbt = _bd(B32)
    lct = _bd(C32)
    obd = _bd(np.ones((64, 64), np.float32))
    v0 = np.concatenate([_v0_vec(), _v0_vec()]).reshape(128, 1)

    xT = np.ascontiguousarray(np.asarray(x, np.float32).T)  # (64, BATCH)
    yT = np.ascontiguousarray(np.asarray(y, np.float32).T)

    in_maps = []
    for c in range(NCORES):
        b0 = c * SHARD
        xc = np.concatenate(
            [xT[:, b0 : b0 + NFREE], xT[:, b0 + NFREE : b0 + SHARD]], axis=0
        )
        yc = np.concatenate(
            [yT[:, b0 : b0 + NFREE], yT[:, b0 + NFREE : b0 + SHARD]], axis=0
        )
        in_maps.append(
            {
                "x": np.ascontiguousarray(xc),
                "y": np.ascontiguousarray(yc),
                "lms": lms, "lbf": lbf, "lcf": lcf, "lbt": lbt, "lct": lct,
                "obd": obd, "v0": v0,
            }
        )

    res = run_bass_kernel_spmd(nc, in_maps, core_ids=list(range(NCORES)))
    out = np.concatenate([res.results[c]["lam"] for c in range(NCORES)])
    return out.reshape(BATCH, 1).astype(np.float32)

